# revision 3
# baseline (speedup 1.0000x reference)
"""ConvLSTM (Conv1D-LSTM over frames, sequential in time) on 8 NeuronCores.

Data-parallel over batch (8 per core). Per core the LSTM state is kept
transposed as (C=64, batch*frame) in SBUF so the recurrent 1-D conv becomes
PSUM-accumulated float32r matmuls with no per-step transpose.

v2 restructure (engine rebalance around the 40-matmul/step PE floor):
  - 4 recurrent chains (2 batches each); gates for a chain computed as two
    128-col PSUM halves of one [128,1024] tile (one K=74 [h-shift8;
    z-window; ones] matmul first, then 4 paired-tap matmuls per half).
  - ACT load split: chains 0/1 use tanh(j) on ACT; chains 2/3 fold j into a
    single merged sigmoid over [128,1024] (weights for j pre-scaled by 2,
    tanh(j) = 2*sigmoid(2j)-1 recovered by one DVE tensor_scalar).
  - per-chain gate column order alternates ([f,i,o,j] / [i,f,j,o]) so every
    elementwise op has same-base-partition inputs; the c' add and tanh(c')
    are pair-merged into single 128-partition ops.
  - h' multiply and the channel mean (gpsimd partition_all_reduce) run on
    Pool; the K=74 h-slice feed is an SBUF-to-SBUF DMA with a full step of
    slack; z windows and mean flushes are DMA-batched over 2-step bands.
"""
import sys
from contextlib import ExitStack

import numpy as np

if '/opt/trn_rl_repo' not in sys.path:
    sys.path.insert(0, '/opt/trn_rl_repo')

import concourse.bacc as bacc
import concourse.tile as tile
from concourse import bass_isa, mybir
from concourse.bass_utils import run_bass_kernel_spmd

B, T, F, C = 64, 64, 256, 64
NCORES, BL = 8, 8            # batches per core
NG = 4                       # groups (chains) per core, 2 batches each
COLS = F + 8                 # padded frame axis (4 each side)
W = 2 * F                    # free width per group (2 batches x 256)
F32 = mybir.dt.float32
F32R = mybir.dt.float32r
ACTF = mybir.ActivationFunctionType
ALU = mybir.AluOpType

_CACHE = {}


def _prep_weights(Wx, Wh, b):
    """Per-group packed weights (NG, 128, 5, 256).

    Source gate order (reference): i, j, f, o. Per-group column orders:
      g0: [f, i, o, j]      g1: [i, f, j, o]
      g2: [f, i, o, 2*j]    g3: [i, f, 2*j, o]
    Forget bias (+1.0) folded into the bias row; j columns of Wh/Wx/bias
    scaled by 2 for groups 2/3 (tanh(j) = 2*sigmoid(2j)-1).
    """
    iS, jS, fS, oS = (np.arange(0, 64), np.arange(64, 128),
                      np.arange(128, 192), np.arange(192, 256))
    orders = [
        np.concatenate([fS, iS, oS, jS]),   # g0
        np.concatenate([iS, fS, jS, oS]),   # g1
        np.concatenate([fS, iS, oS, jS]),   # g2
        np.concatenate([iS, fS, jS, oS]),   # g3
    ]
    fpos = [np.arange(0, 64), np.arange(64, 128),
            np.arange(0, 64), np.arange(64, 128)]
    jpos = [np.arange(192, 256), np.arange(128, 192),
            np.arange(192, 256), np.arange(128, 192)]

    out = []
    for g in range(NG):
        perm = orders[g]
        Whp = Wh[:, :, perm].astype(np.float32).copy()     # (9, 64, 256)
        Wxp = Wx[:, 0, perm].astype(np.float32).copy()     # (9, 256)
        bp = b[perm].astype(np.float32).copy()
        bp[fpos[g]] += 1.0                                  # forget-gate bias
        if g >= 2:
            Whp[:, :, jpos[g]] *= 2.0
            Wxp[:, jpos[g]] *= 2.0
            bp[jpos[g]] *= 2.0
        whall = np.zeros((128, 5, 256), np.float32)
        for p in range(4):
            whall[0:64, p] = Whp[2 * p]
            whall[64:128, p] = Whp[2 * p + 1]
        whall[0:64, 4] = Whp[8]
        whall[64:73, 4] = Wxp
        whall[73, 4] = bp
        out.append(whall)
    return np.ascontiguousarray(np.stack(out))             # (NG, 128, 5, 256)


def _prep_core(z, h0, c0, core):
    zc = z[BL * core:BL * core + BL, :, :, 0]          # (8, T, F)
    h0c = h0[BL * core:BL * core + BL]                 # (8, F, C)
    c0c = c0[BL * core:BL * core + BL]

    zp = np.zeros((BL, T, COLS), np.float32)
    zp[:, :, 4:260] = zc
    # z windows batched per 2-step band: (T//2, NG, 10, 2, 2, 256)
    zpa = np.ones((T // 2, NG, 10, 2, 2, 256), np.float32)
    for g in range(NG):
        for bb in range(2):
            bidx = 2 * g + bb
            for k in range(9):
                for s in range(2):
                    zpa[:, g, k, s, bb, :] = zp[bidx, s::2, k:k + 256]

    h0T = np.ascontiguousarray(h0c.transpose(2, 0, 1)).astype(np.float32)
    hh0 = np.zeros((2, NG, 128, 2, COLS), np.float32)
    for g in range(NG):
        for bb in range(2):
            hh0[0, g, 0:64, bb, 4:260] = h0T[:, 2 * g + bb, :]
    hh0[0, :, 64:128, :, 0:COLS - 1] = hh0[0, :, 0:64, :, 1:COLS]

    # c0 packed per pair: rows 0:64 = even group, 64:128 = odd group
    c0a = np.zeros((2, 128, 2, 256), np.float32)
    for g in range(NG):
        for bb in range(2):
            c0a[g // 2, 64 * (g % 2):64 * (g % 2) + 64, bb] = c0c[2 * g + bb].T

    return {
        'zpa': np.ascontiguousarray(zpa),
        'hh0': np.ascontiguousarray(hh0.reshape(2, NG, 128, 2 * COLS)),
        'c0a': np.ascontiguousarray(c0a.reshape(2, 128, W)),
    }


def _build_program():
    nc = bacc.Bacc("TRN2", target_bir_lowering=False, debug=False,
                   enable_asserts=True, num_devices=NCORES)
    zpa_d = nc.dram_tensor("zpa", (T // 2, NG, 10, 2, 2, 256), F32R,
                           kind="ExternalInput")
    hh0_d = nc.dram_tensor("hh0", (2, NG, 128, 2 * COLS), F32R,
                           kind="ExternalInput")
    c0a_d = nc.dram_tensor("c0a", (2, 128, W), F32, kind="ExternalInput")
    wh_d = nc.dram_tensor("whall", (NG, 128, 5, 256), F32R,
                          kind="ExternalInput")
    out_d = nc.dram_tensor("out", (64, 2, 2, 2, 256), F32, kind="ExternalOutput")

    with tile.TileContext(nc) as tc, ExitStack() as ctx:
        consts = ctx.enter_context(tc.tile_pool(name="consts", bufs=1))
        state = ctx.enter_context(tc.tile_pool(name="state", bufs=1))
        work = ctx.enter_context(tc.tile_pool(name="work", bufs=2))
        pgA_pool = ctx.enter_context(tc.tile_pool(name="pgA", bufs=2,
                                                  space="PSUM"))
        pgB_pool = ctx.enter_context(tc.tile_pool(name="pgB", bufs=2,
                                                  space="PSUM"))

        wh_t = consts.tile([128, NG, 5, 256], F32R)
        nc.sync.dma_start(out=wh_t[:], in_=wh_d[:])

        hh = [[state.tile([128, 2, COLS], F32R, name=f"hh{par}{g}",
                          tag=f"hh{par}{g}")
               for g in range(NG)] for par in range(2)]
        CC = [[state.tile([128, W], F32, name=f"CC{par}{p}", tag=f"CC{par}{p}")
               for p in range(2)] for par in range(2)]
        # K=74 rhs: rows 0:64 h-shift8 (per step slot), 64:74 z+ones (band)
        r8 = [[state.tile([74, 2, 2, 256], F32R, name=f"r8{bb}{g}",
                          tag=f"r8{bb}{g}")
               for g in range(NG)] for bb in range(2)]
        srow = [[state.tile([64, 2, W], F32, name=f"sr{bb}{g}",
                            tag=f"sr{bb}{g}")
                 for g in range(NG)] for bb in range(2)]

        for par in range(2):
            for g in range(NG):
                nc.sync.dma_start(out=hh[par][g][:], in_=hh0_d[par, g])
        for p in range(2):
            nc.sync.dma_start(out=CC[0][p][:], in_=c0a_d[p])

        # prologue: z bands 0/1; h-shift8 slot 0 from the initial hh
        for g in range(NG):
            nc.sync.dma_start(out=r8[0][g][64:74], in_=zpa_d[0, g])
            nc.sync.dma_start(out=r8[1][g][64:74], in_=zpa_d[1, g])
            nc.sync.dma_start(out=r8[0][g][0:64, 0],
                              in_=hh[0][g][0:64, :, 8:264])

        outs_sb = state.tile([128, 1024], F32, name="outs_sb", tag="outs_sb")

        for t in range(T):
            par, npar = t % 2, (t + 1) % 2
            band, slot = (t // 2) % 2, t % 2
            # prefetch z for the band after the current one
            if slot == 0 and 2 <= t and t + 2 < T:
                for g in range(NG):
                    nc.sync.dma_start(out=r8[(band + 1) % 2][g][64:74],
                                      in_=zpa_d[t // 2 + 1, g])

            SGm = [None] * NG   # merged sigmoid (g2/g3)
            SG0 = [None] * NG   # sigma(P0) for tanh-groups
            TJ = [None] * 2     # tanh(j) per pair
            SO = [None] * 2     # sigma(o) per pair (tanh-groups)
            Apair = [None] * 2
            Bpair = [None] * 2

            for g in range(NG):
                hcur = hh[par][g]
                pr = g // 2
                if g < 2:
                    P = pgA_pool.tile([128, 1024], F32, name=f"PA{g}",
                                      tag="pga")
                else:
                    P = pgB_pool.tile([128, 1024], F32, name=f"PB{g}",
                                      tag="pgb")
                for m in range(2):
                    nc.tensor.matmul(
                        P[:, 512 * m:512 * m + 512],
                        wh_t[0:74, g, 4, m * 128:(m + 1) * 128],
                        r8[band][g][0:74, slot], start=True, stop=False)
                    for tap in range(4):
                        nc.tensor.matmul(
                            P[:, 512 * m:512 * m + 512],
                            wh_t[:, g, tap, m * 128:(m + 1) * 128],
                            hcur[:, :, 2 * tap:2 * tap + 256],
                            start=False, stop=(tap == 3))

                if g < 2:
                    sg0 = work.tile([128, W], F32, name=f"SG0{g}",
                                    tag=f"sg0{g}")
                    SG0[g] = sg0
                    nc.scalar.activation(out=sg0[:], in_=P[:, 0:512],
                                         func=ACTF.Sigmoid)
                    if TJ[pr] is None:
                        TJ[pr] = work.tile([128, W], F32, name=f"TJ{pr}",
                                           tag=f"tj{pr}")
                        SO[pr] = work.tile([128, W], F32, name=f"SO{pr}",
                                           tag=f"so{pr}")
                    jhi, ohi = (64, 0) if g % 2 == 0 else (0, 64)
                    nc.scalar.activation(
                        out=TJ[pr][jhi:jhi + 64, :],
                        in_=P[jhi:jhi + 64, 512:1024], func=ACTF.Tanh)
                    nc.scalar.activation(
                        out=SO[pr][ohi:ohi + 64, :],
                        in_=P[ohi:ohi + 64, 512:1024], func=ACTF.Sigmoid)
                else:
                    sgm = work.tile([128, 1024], F32, name=f"SGm{g}",
                                    tag=f"sgm{g}")
                    SGm[g] = sgm
                    nc.scalar.activation(out=sgm[:], in_=P[:],
                                         func=ACTF.Sigmoid)
                    if TJ[pr] is None:
                        TJ[pr] = work.tile([128, W], F32, name=f"TJb{pr}",
                                           tag=f"tjb{pr}")
                    jhi = 64 * ((g + 1) % 2)   # g2: j at 64; g3: j at 0
                    nc.vector.tensor_scalar(
                        TJ[pr][jhi:jhi + 64, :],
                        sgm[jhi:jhi + 64, 512:1024],
                        2.0, 1.0, ALU.mult, ALU.subtract)

            for g in range(NG):
                hi = 64 * (g % 2)
                pr = g // 2
                sg = SG0[g] if g < 2 else SGm[g]
                if Apair[pr] is None:
                    Apair[pr] = work.tile([128, W], F32, name=f"A{pr}",
                                          tag=f"a{pr}")
                    Bpair[pr] = work.tile([128, W], F32, name=f"B{pr}",
                                          tag=f"b{pr}")
                # A = c * sigma(f)   (inputs and output at base hi)
                nc.vector.tensor_mul(Apair[pr][hi:hi + 64, :],
                                     CC[par][pr][hi:hi + 64, :],
                                     sg[hi:hi + 64, 0:512])
                # B = sigma(i) * tanh(j)  (inputs at 64-hi, output at hi)
                ihi = 64 - hi
                nc.vector.tensor_mul(Bpair[pr][hi:hi + 64, :],
                                     sg[ihi:ihi + 64, 0:512],
                                     TJ[pr][ihi:ihi + 64, :])

            TCn = [None] * 2
            for pr in range(2):
                # c' = A + B and tanh(c'), both pair-merged (128 partitions)
                nc.vector.tensor_add(CC[npar][pr][:], Apair[pr][:],
                                     Bpair[pr][:])
                TCn[pr] = work.tile([128, W], F32, name=f"TC{pr}",
                                    tag=f"tcp{pr}")
                nc.scalar.activation(out=TCn[pr][:], in_=CC[npar][pr][:],
                                     func=ACTF.Tanh)

            for g in range(NG):
                hi = 64 * (g % 2)
                pr = g // 2
                hnext = hh[npar][g]
                if g < 2:
                    so_ap = SO[pr][hi:hi + 64, :]
                else:
                    so_ap = SGm[g][hi:hi + 64, 512:1024]
                # h' = tanh(c') * sigma(o)  -> hh rows 0:64 (Pool)
                nc.gpsimd.tensor_mul(hnext[0:64, :, 4:260],
                                     TCn[pr][hi:hi + 64, :], so_ap)
                # channel mean via gpsimd partition all-reduce
                nc.gpsimd.partition_all_reduce(
                    srow[band][g][:, slot, :], hnext[0:64, :, 4:260],
                    channels=64, reduce_op=bass_isa.ReduceOp.add)
                # shift-by-1 copy (rows 64:128) on DVE
                nc.vector.tensor_copy(out=hnext[64:128, :, 0:263],
                                      in_=hnext[0:64, :, 1:264])
                # h-shift8 slice for the next step's K=74 matmul (DMA; a
                # full step of slack before it is consumed)
                if t + 1 < T:
                    nb, ns = ((t + 1) // 2) % 2, (t + 1) % 2
                    nc.sync.dma_start(out=r8[nb][g][0:64, ns],
                                      in_=hnext[0:64, :, 8:264])
                # flush means every 2 steps
                if slot == 1:
                    r0 = 64 * (g // 2) + t - 1
                    nc.sync.dma_start(
                        out=outs_sb[r0:r0 + 2, (g % 2) * W:(g % 2) * W + W],
                        in_=srow[band][g][0:1, :, :])

        outs_tb = consts.tile([128, 1024], F32)
        nc.scalar.activation(out=outs_tb[:], in_=outs_sb[:], func=ACTF.Tanh,
                             scale=1.0 / 64.0)
        for gh in range(2):
            nc.sync.dma_start(out=out_d[:, gh],
                              in_=outs_tb[64 * gh:64 * gh + 64, :])

    nc.compile()
    return nc


def _get_program():
    if 'nc' not in _CACHE:
        _CACHE['nc'] = _build_program()
    return _CACHE['nc']


def kernel(z, h0, c0, Wx, Wh, b):
    z = np.asarray(z, np.float32)
    h0 = np.asarray(h0, np.float32)
    c0 = np.asarray(c0, np.float32)
    whall = _prep_weights(np.asarray(Wx, np.float32),
                          np.asarray(Wh, np.float32),
                          np.asarray(b, np.float32))
    in_maps = []
    for core in range(NCORES):
        m = _prep_core(z, h0, c0, core)
        m['whall'] = whall
        in_maps.append(m)
    nc = _get_program()
    res = run_bass_kernel_spmd(nc, in_maps, list(range(NCORES)))
    outs = []
    for core in range(NCORES):
        R = res.results[core]['out']        # (64, 2, 2, 2, 256) [t,gh,gl,bb,f]
        outs.append(R.transpose(1, 2, 3, 0, 4).reshape(BL, T * F))
    return np.concatenate(outs, axis=0)


# revision 5
# speedup vs baseline: 1.4844x; 1.4844x over previous
"""ConvLSTM (Conv1D-LSTM over frames, sequential in time) on 8 NeuronCores.

Data-parallel over batch (8 per core). Per core the LSTM state is kept
transposed as (C=64, batch*frame) in SBUF so the recurrent 1-D conv becomes
PSUM-accumulated float32r matmuls with no per-step transpose:

  - 4 independent recurrent chains per core (one 2-batch group each, 512 gate
    columns) keep the PE densely fed (HAM-warm) and hide the per-step
    dependency chain.
  - h is stored padded (264 cols/batch); rows 64:128 hold a shift-by-1 copy
    so two conv taps contract per K=128 matmul (4 paired-tap matmuls).
  - the 9th tap, the z-conv (Cin=1), the bias, and the forget bias fold into
    one K=74 matmul over [h-slice; z-sliding-window; ones].

Engine rebalance vs the first version (Pool was at 94%, ACT 91%):
  - the K=74 h-slice copy is one SBUF-to-SBUF DMA (was DVE+Pool copies);
    it reads the previous step's h so the DMA latency is fully hidden.
  - groups 2/3 fold tanh(j) into the [j;o] sigmoid (j weights pre-scaled by
    2; tanh(j)=2*sigmoid(2j)-1 via one DVE tensor_scalar), saving one ACT op
    per group-step, and run the h'-multiply on Pool instead of DVE.
  - groups 0/1 compute the channel mean as a ones-vector PE matmul into a
    PSUM bank (emitted one step late so it never blocks gate matmuls in the
    in-order PE queue); groups 2/3 keep gpsimd partition_all_reduce.
"""
import sys
from contextlib import ExitStack

import numpy as np

if '/opt/trn_rl_repo' not in sys.path:
    sys.path.insert(0, '/opt/trn_rl_repo')

import concourse.bacc as bacc
import concourse.tile as tile
from concourse import bass_isa, mybir
from concourse.bass_utils import run_bass_kernel_spmd

B, T, F, C = 64, 64, 256, 64
NCORES, BL = 8, 8            # batches per core
NG = 4                       # groups (chains) per core, 2 batches each
COLS = F + 8                 # padded frame axis (4 each side)
W = 2 * F                    # free width per group (2 batches x 256)
F32 = mybir.dt.float32
F32R = mybir.dt.float32r
ACTF = mybir.ActivationFunctionType
ALU = mybir.AluOpType

_CACHE = {}


def _prep_weights(Wx, Wh, b):
    # gate reorder: [f, i, j, o]  (reference order: i, j, f, o)
    perm = np.concatenate([np.arange(128, 192), np.arange(0, 64),
                           np.arange(64, 128), np.arange(192, 256)])
    Whp = Wh[:, :, perm].astype(np.float32)            # (9, 64, 256)
    Wxp = Wx[:, 0, perm].astype(np.float32)            # (9, 256)
    bp = b[perm].astype(np.float32).copy()
    bp[0:64] += 1.0                                    # forget-gate bias
    whall = np.zeros((2, 128, 5, 256), np.float32)
    for p in range(4):
        whall[0, 0:64, p] = Whp[2 * p]
        whall[0, 64:128, p] = Whp[2 * p + 1]
    whall[0, 0:64, 4] = Whp[8]
    whall[0, 64:73, 4] = Wxp
    whall[0, 73, 4] = bp
    # variant 1 (groups 2/3): j columns scaled by 2 for the sigmoid(2j) trick
    whall[1] = whall[0]
    whall[1, :, :, 128:192] *= 2.0
    return np.ascontiguousarray(whall)


def _prep_core(z, h0, c0, core):
    zc = z[BL * core:BL * core + BL, :, :, 0]          # (8, T, F)
    h0c = h0[BL * core:BL * core + BL]                 # (8, F, C)
    c0c = c0[BL * core:BL * core + BL]

    zp = np.zeros((BL, T, COLS), np.float32)
    zp[:, :, 4:260] = zc
    zpa = np.ones((T, NG, 10, 2, 256), np.float32)
    for g in range(NG):
        for bb in range(2):
            bidx = 2 * g + bb
            for k in range(9):
                zpa[:, g, k, bb, :] = zp[bidx, :, k:k + 256]

    h0T = np.ascontiguousarray(h0c.transpose(2, 0, 1)).astype(np.float32)
    hh0 = np.zeros((2, NG, 128, 2, COLS), np.float32)
    for g in range(NG):
        for bb in range(2):
            hh0[0, g, 0:64, bb, 4:260] = h0T[:, 2 * g + bb, :]
    hh0[0, :, 64:128, :, 0:COLS - 1] = hh0[0, :, 0:64, :, 1:COLS]

    c0a = np.zeros((NG, 64, 2, 256), np.float32)
    for g in range(NG):
        for bb in range(2):
            c0a[g, :, bb, :] = c0c[2 * g + bb].T

    return {
        'zpa': np.ascontiguousarray(zpa.reshape(T, NG, 10, W)),
        'hh0': np.ascontiguousarray(hh0.reshape(2, NG, 128, 2 * COLS)),
        'c0a': np.ascontiguousarray(c0a.reshape(NG, 64, W)),
    }


def _build_program():
    nc = bacc.Bacc("TRN2", target_bir_lowering=False, debug=False,
                   enable_asserts=True, num_devices=NCORES)
    zpa_d = nc.dram_tensor("zpa", (T, NG, 10, W), F32R, kind="ExternalInput")
    hh0_d = nc.dram_tensor("hh0", (2, NG, 128, 2 * COLS), F32R,
                           kind="ExternalInput")
    c0a_d = nc.dram_tensor("c0a", (NG, 64, W), F32, kind="ExternalInput")
    wh_d = nc.dram_tensor("whall", (2, 128, 5, 256), F32R,
                          kind="ExternalInput")
    out_d = nc.dram_tensor("out", (64, 2, 2, 2, 256), F32, kind="ExternalOutput")

    with tile.TileContext(nc) as tc, ExitStack() as ctx:
        consts = ctx.enter_context(tc.tile_pool(name="consts", bufs=1))
        state = ctx.enter_context(tc.tile_pool(name="state", bufs=1))
        y_pool = ctx.enter_context(tc.tile_pool(name="ypool", bufs=4))
        ts_pool = ctx.enter_context(tc.tile_pool(name="tspool", bufs=4))
        m_pool = ctx.enter_context(tc.tile_pool(name="mpool", bufs=4))
        r8_pool = ctx.enter_context(tc.tile_pool(name="r8pool", bufs=12))
        srow_pool = ctx.enter_context(tc.tile_pool(name="srowpool", bufs=8))
        pg_pool = ctx.enter_context(tc.tile_pool(name="pgpool", bufs=7,
                                                 space="PSUM"))
        mb_pool = ctx.enter_context(tc.tile_pool(name="mbpool", bufs=1,
                                                 space="PSUM"))
        outs_pool = ctx.enter_context(tc.tile_pool(name="outs", bufs=1))

        wh_t = consts.tile([128, 2, 5, 256], F32R)
        nc.sync.dma_start(out=wh_t[:], in_=wh_d[:])
        ones64 = consts.tile([64, 1], F32R)
        nc.vector.memset(ones64[:], 1.0)

        hh = [[state.tile([128, 2, COLS], F32R, name=f"hh{par}{g}",
                          tag=f"hh{par}{g}")
               for g in range(NG)] for par in range(2)]
        # CJ[g]: rows 0:64 = c state (persistent), rows 64:128 = tanh(j)
        CJ = [state.tile([128, W], F32, name=f"CJ{g}", tag=f"CJ{g}")
              for g in range(NG)]
        for par in range(2):
            for g in range(NG):
                nc.sync.dma_start(out=hh[par][g][:], in_=hh0_d[par, g])
        for g in range(NG):
            nc.sync.dma_start(out=CJ[g][0:64, :], in_=c0a_d[g])

        outs_sb = outs_pool.tile([128, 1024], F32)
        MB = mb_pool.tile([128, 512], F32)   # PE-mean bank (groups 0/1)

        def pe_means(t, par):
            # channel means for groups 0/1 of step t, read from hh[par]
            # (emitted at the top of step t+1 so they never stall the
            # in-order PE queue behind unfinished h updates)
            for g in range(2):
                nc.tensor.matmul(MB[32 * g:32 * g + 1, :], ones64[:],
                                 hh[par][g][0:64, :, 4:260],
                                 start=True, stop=True,
                                 tile_position=(0, 32 * g))
            sr2 = srow_pool.tile([2, W], F32, name="sr2", tag="sr2")
            nc.vector.tensor_copy(out=sr2[:], in_=MB[0:64:32, :])
            nc.sync.dma_start(out=outs_sb[t:t + 1, 0:1024], in_=sr2[:])

        for t in range(T):
            par, npar = t % 2, (t + 1) % 2
            if t > 0:
                pe_means(t - 1, par)
            for g in range(NG):
                wv = 0 if g < 2 else 1
                hcur, hnext = hh[par][g], hh[npar][g]
                r8 = r8_pool.tile([80, 2, 256], F32R)
                nc.sync.dma_start(out=r8[0:64, :, :],
                                  in_=hcur[0:64, :, 8:264])
                nc.sync.dma_start(out=r8[64:74, :, :], in_=zpa_d[t, g])

                # P1 ([j; o]) first: its consumers get a head start.
                P1 = pg_pool.tile([128, W], F32, name="P1", tag="pg")
                P0 = pg_pool.tile([128, W], F32, name="P0", tag="pg")
                for m, P in ((1, P1), (0, P0)):
                    for tap in range(4):
                        nc.tensor.matmul(
                            P[:], wh_t[:, wv, tap, m * 128:(m + 1) * 128],
                            hcur[:, :, 2 * tap:2 * tap + 256],
                            start=(tap == 0), stop=False)
                    nc.tensor.matmul(
                        P[:], wh_t[0:74, wv, 4, m * 128:(m + 1) * 128],
                        r8[0:74, :, :], start=False, stop=True)

                S = y_pool.tile([128, W], F32)
                if g < 2:
                    # tanh-j path: S = [sig f; sig i]; CJ[64:] = tanh j;
                    # TO = [tanh c | sig o] on rows 0:64
                    TO = ts_pool.tile([64, 2 * W], F32)
                    nc.scalar.activation(out=CJ[g][64:128, :],
                                         in_=P1[0:64, :], func=ACTF.Tanh)
                    nc.scalar.activation(out=TO[:, W:2 * W],
                                         in_=P1[64:128, :],
                                         func=ACTF.Sigmoid)
                    nc.scalar.activation(out=S[:], in_=P0[:],
                                         func=ACTF.Sigmoid)
                    MM = m_pool.tile([64, 2 * W], F32)
                    nc.gpsimd.tensor_mul(MM[:, W:2 * W], S[64:128, :],
                                         CJ[g][64:128, :])
                    nc.vector.tensor_mul(MM[:, 0:W], S[0:64, :],
                                         CJ[g][0:64, :])
                    nc.vector.tensor_add(CJ[g][0:64, :], MM[:, 0:W],
                                         MM[:, W:2 * W])
                    nc.scalar.activation(out=TO[:, 0:W], in_=CJ[g][0:64, :],
                                         func=ACTF.Tanh)
                    nc.vector.tensor_mul(hnext[0:64, :, 4:260],
                                         TO[:, 0:W], TO[:, W:2 * W])
                    nc.vector.tensor_copy(out=hnext[64:128, :, 3:259],
                                          in_=hnext[0:64, :, 4:260])
                    # channel mean via PE ones-matmul, emitted next step
                else:
                    # sigmoid(2j) path: S1 = sig([2j; o]) in one ACT op;
                    # tanh j = 2*sig(2j)-1 on DVE; tanh c written to rows
                    # 64:128 so the Pool h'-mul has same-base inputs.
                    S1 = y_pool.tile([128, W], F32, name="S1", tag="s1")
                    T2 = ts_pool.tile([128, W], F32, name="T2", tag="t2")
                    nc.scalar.activation(out=S1[:], in_=P1[:],
                                         func=ACTF.Sigmoid)
                    nc.vector.tensor_scalar(CJ[g][64:128, :], S1[0:64, :],
                                            2.0, 1.0, ALU.mult, ALU.subtract)
                    nc.scalar.activation(out=S[:], in_=P0[:],
                                         func=ACTF.Sigmoid)
                    MM = m_pool.tile([64, 2 * W], F32)
                    nc.gpsimd.tensor_mul(MM[:, W:2 * W], S[64:128, :],
                                         CJ[g][64:128, :])
                    nc.vector.tensor_mul(MM[:, 0:W], S[0:64, :],
                                         CJ[g][0:64, :])
                    nc.vector.tensor_add(CJ[g][0:64, :], MM[:, 0:W],
                                         MM[:, W:2 * W])
                    nc.scalar.activation(out=T2[64:128, :],
                                         in_=CJ[g][0:64, :], func=ACTF.Tanh)
                    nc.gpsimd.tensor_mul(hnext[0:64, :, 4:260],
                                         T2[64:128, :], S1[64:128, :])
                    nc.vector.tensor_copy(out=hnext[64:128, :, 3:259],
                                          in_=hnext[0:64, :, 4:260])
                    srow = srow_pool.tile([64, W], F32)
                    nc.gpsimd.partition_all_reduce(
                        srow[:], hnext[0:64, :, 4:260], channels=64,
                        reduce_op=bass_isa.ReduceOp.add)
                    nc.sync.dma_start(
                        out=outs_sb[64 + t:64 + t + 1,
                                    (g % 2) * W:(g % 2) * W + W],
                        in_=srow[0:1, :])

        pe_means(T - 1, T % 2)

        outs_tb = consts.tile([128, 1024], F32)
        nc.scalar.activation(out=outs_tb[:], in_=outs_sb[:], func=ACTF.Tanh,
                             scale=1.0 / 64.0)
        for gh in range(2):
            nc.sync.dma_start(out=out_d[:, gh],
                              in_=outs_tb[64 * gh:64 * gh + 64, :])

    nc.compile()
    return nc


def _get_program():
    if 'nc' not in _CACHE:
        _CACHE['nc'] = _build_program()
    return _CACHE['nc']


def kernel(z, h0, c0, Wx, Wh, b):
    z = np.asarray(z, np.float32)
    h0 = np.asarray(h0, np.float32)
    c0 = np.asarray(c0, np.float32)
    whall = _prep_weights(np.asarray(Wx, np.float32),
                          np.asarray(Wh, np.float32),
                          np.asarray(b, np.float32))
    in_maps = []
    for core in range(NCORES):
        m = _prep_core(z, h0, c0, core)
        m['whall'] = whall
        in_maps.append(m)
    nc = _get_program()
    res = run_bass_kernel_spmd(nc, in_maps, list(range(NCORES)))
    outs = []
    for core in range(NCORES):
        R = res.results[core]['out']        # (64, 2, 2, 2, 256) [t,gh,gl,bb,f]
        outs.append(R.transpose(1, 2, 3, 0, 4).reshape(BL, T * F))
    return np.concatenate(outs, axis=0)


# revision 8
# speedup vs baseline: 1.5344x; 1.0337x over previous
"""ConvLSTM (Conv1D-LSTM over frames, sequential in time) on 8 NeuronCores.

Data-parallel over batch (8 per core). Per core the LSTM state is kept
transposed as (C=64, batch*frame) in SBUF so the recurrent 1-D conv becomes
PSUM-accumulated float32r matmuls with no per-step transpose:

  - 4 independent recurrent chains per core (one 2-batch group each, 512 gate
    columns) keep the PE densely fed (HAM-warm) and hide the per-step
    dependency chain.
  - h is stored padded (264 cols/batch); rows 64:128 hold a shift-by-1 copy
    so two conv taps contract per K=128 matmul (4 paired-tap matmuls).
  - the 9th tap, the z-conv (Cin=1), the bias, and the forget bias fold into
    one K=74 matmul over [h-slice; z-sliding-window; ones].

Engine rebalance vs the first version (Pool was at 94%, ACT 91%):
  - the K=74 h-slice copy is one SBUF-to-SBUF DMA (was DVE+Pool copies);
    it reads the previous step's h so the DMA latency is fully hidden.
  - groups 2/3 fold tanh(j) into the [j;o] sigmoid (j weights pre-scaled by
    2; tanh(j)=2*sigmoid(2j)-1 via one DVE tensor_scalar), saving one ACT op
    per group-step, and run the h'-multiply on Pool instead of DVE.
  - groups 0/1 compute the channel mean as a ones-vector PE matmul into a
    PSUM bank (emitted one step late so it never blocks gate matmuls in the
    in-order PE queue); groups 2/3 keep gpsimd partition_all_reduce.
"""
import sys
from contextlib import ExitStack

import numpy as np

if '/opt/trn_rl_repo' not in sys.path:
    sys.path.insert(0, '/opt/trn_rl_repo')

import concourse.bacc as bacc
import concourse.tile as tile
from concourse import bass_isa, mybir
from concourse.bass_utils import run_bass_kernel_spmd

B, T, F, C = 64, 64, 256, 64
NCORES, BL = 8, 8            # batches per core
NG = 4                       # groups (chains) per core, 2 batches each
COLS = F + 8                 # padded frame axis (4 each side)
W = 2 * F                    # free width per group (2 batches x 256)
F32 = mybir.dt.float32
F32R = mybir.dt.float32r
ACTF = mybir.ActivationFunctionType
ALU = mybir.AluOpType

_CACHE = {}


def _prep_weights(Wx, Wh, b):
    # gate reorder: [f, i, j, o]  (reference order: i, j, f, o)
    perm = np.concatenate([np.arange(128, 192), np.arange(0, 64),
                           np.arange(64, 128), np.arange(192, 256)])
    Whp = Wh[:, :, perm].astype(np.float32)            # (9, 64, 256)
    Wxp = Wx[:, 0, perm].astype(np.float32)            # (9, 256)
    bp = b[perm].astype(np.float32).copy()
    bp[0:64] += 1.0                                    # forget-gate bias
    whall = np.zeros((2, 128, 5, 256), np.float32)
    for p in range(4):
        whall[0, 0:64, p] = Whp[2 * p]
        whall[0, 64:128, p] = Whp[2 * p + 1]
    whall[0, 0:64, 4] = Whp[8]
    whall[0, 64:73, 4] = Wxp
    whall[0, 73, 4] = bp
    # variant 1 (groups 2/3): j columns scaled by 2 for the sigmoid(2j) trick
    whall[1] = whall[0]
    whall[1, :, :, 128:192] *= 2.0
    return np.ascontiguousarray(whall)


def _prep_core(z, h0, c0, core):
    zc = z[BL * core:BL * core + BL, :, :, 0]          # (8, T, F)
    h0c = h0[BL * core:BL * core + BL]                 # (8, F, C)
    c0c = c0[BL * core:BL * core + BL]

    zp = np.zeros((BL, T, COLS), np.float32)
    zp[:, :, 4:260] = zc
    zpa = np.ones((T, NG, 10, 2, 256), np.float32)
    for g in range(NG):
        for bb in range(2):
            bidx = 2 * g + bb
            for k in range(9):
                zpa[:, g, k, bb, :] = zp[bidx, :, k:k + 256]

    h0T = np.ascontiguousarray(h0c.transpose(2, 0, 1)).astype(np.float32)
    hh0 = np.zeros((2, NG, 128, 2, COLS), np.float32)
    for g in range(NG):
        for bb in range(2):
            hh0[0, g, 0:64, bb, 4:260] = h0T[:, 2 * g + bb, :]
    hh0[0, :, 64:128, :, 0:COLS - 1] = hh0[0, :, 0:64, :, 1:COLS]

    c0a = np.zeros((NG, 64, 2, 256), np.float32)
    for g in range(NG):
        for bb in range(2):
            c0a[g, :, bb, :] = c0c[2 * g + bb].T

    return {
        'zpa': np.ascontiguousarray(zpa.reshape(T, NG, 10, W)),
        'hh0': np.ascontiguousarray(hh0.reshape(2, NG, 128, 2 * COLS)),
        'c0a': np.ascontiguousarray(c0a.reshape(NG, 64, W)),
    }


def _build_program():
    nc = bacc.Bacc("TRN2", target_bir_lowering=False, debug=False,
                   enable_asserts=True, num_devices=NCORES)
    zpa_d = nc.dram_tensor("zpa", (T, NG, 10, W), F32R, kind="ExternalInput")
    hh0_d = nc.dram_tensor("hh0", (2, NG, 128, 2 * COLS), F32R,
                           kind="ExternalInput")
    c0a_d = nc.dram_tensor("c0a", (NG, 64, W), F32, kind="ExternalInput")
    wh_d = nc.dram_tensor("whall", (2, 128, 5, 256), F32R,
                          kind="ExternalInput")
    out_d = nc.dram_tensor("out", (64, 2, 2, 2, 256), F32, kind="ExternalOutput")

    with tile.TileContext(nc) as tc, ExitStack() as ctx:
        consts = ctx.enter_context(tc.tile_pool(name="consts", bufs=1))
        state = ctx.enter_context(tc.tile_pool(name="state", bufs=1))
        y_pool = ctx.enter_context(tc.tile_pool(name="ypool", bufs=4))
        ts_pool = ctx.enter_context(tc.tile_pool(name="tspool", bufs=4))
        m_pool = ctx.enter_context(tc.tile_pool(name="mpool", bufs=4))
        r8_pool = ctx.enter_context(tc.tile_pool(name="r8pool", bufs=12))
        srow_pool = ctx.enter_context(tc.tile_pool(name="srowpool", bufs=8))
        pg_pool = ctx.enter_context(tc.tile_pool(name="pgpool", bufs=7,
                                                 space="PSUM"))
        mb_pool = ctx.enter_context(tc.tile_pool(name="mbpool", bufs=1,
                                                 space="PSUM"))
        outs_pool = ctx.enter_context(tc.tile_pool(name="outs", bufs=1))

        wh_t = consts.tile([128, 2, 5, 256], F32R)
        nc.sync.dma_start(out=wh_t[:], in_=wh_d[:])
        ones64 = consts.tile([64, 1], F32R)
        nc.vector.memset(ones64[:], 1.0)

        hh = [[state.tile([128, 2, COLS], F32R, name=f"hh{par}{g}",
                          tag=f"hh{par}{g}")
               for g in range(NG)] for par in range(2)]
        # CJ[g]: rows 0:64 = c state (persistent), rows 64:128 = tanh(j)
        CJ = [state.tile([128, W], F32, name=f"CJ{g}", tag=f"CJ{g}")
              for g in range(NG)]
        for par in range(2):
            for g in range(NG):
                nc.sync.dma_start(out=hh[par][g][:], in_=hh0_d[par, g])
        for g in range(NG):
            nc.sync.dma_start(out=CJ[g][0:64, :], in_=c0a_d[g])

        outs_sb = outs_pool.tile([128, 1024], F32)
        MB = mb_pool.tile([128, 512], F32)   # PE-mean bank (groups 0/1)

        def pe_means(t):
            # channel means for groups 0/1 of step t, read from the parity
            # buffer h(t) lives in. Emitted two steps later so the matmuls
            # never stall the in-order PE queue behind fresh h updates.
            for g in range(2):
                nc.tensor.matmul(MB[32 * g:32 * g + 1, :], ones64[:],
                                 hh[(t + 1) % 2][g][0:64, :, 4:260],
                                 start=True, stop=True,
                                 tile_position=(0, 32 * g))
            sr2 = srow_pool.tile([2, W], F32, name="sr2", tag="sr2")
            nc.vector.tensor_copy(out=sr2[:], in_=MB[0:64:32, :])
            nc.sync.dma_start(out=outs_sb[t:t + 1, 0:1024], in_=sr2[:])

        # K=74 rhs tiles for step 0: z windows + h0-slice (dep: hh0 load)
        r8cur = []
        for g in range(NG):
            r8 = r8_pool.tile([80, 2, 256], F32R, name="r8p", tag="r8")
            nc.sync.dma_start(out=r8[64:74, :, :], in_=zpa_d[0, g])
            nc.sync.dma_start(out=r8[0:64, :, :], in_=hh[0][g][0:64, :, 8:264])
            r8cur.append(r8)
        srow_prev = [None] * NG

        for t in range(T):
            par, npar = t % 2, (t + 1) % 2
            if t >= 2:
                pe_means(t - 2)
            for g in range(2, NG):
                if srow_prev[g] is not None:
                    nc.sync.dma_start(
                        out=outs_sb[64 + t - 1:64 + t,
                                    (g % 2) * W:(g % 2) * W + W],
                        in_=srow_prev[g][0:1, :])
            r8nxt = [None] * NG
            for g in range(NG):
                wv = 0 if g < 2 else 1
                hcur, hnext = hh[par][g], hh[npar][g]
                r8 = r8cur[g]

                # P1 ([j; o]) first: its consumers get a head start.
                P1 = pg_pool.tile([128, W], F32, name="P1", tag="pg")
                P0 = pg_pool.tile([128, W], F32, name="P0", tag="pg")
                for m, P in ((1, P1), (0, P0)):
                    for tap in range(4):
                        nc.tensor.matmul(
                            P[:], wh_t[:, wv, tap, m * 128:(m + 1) * 128],
                            hcur[:, :, 2 * tap:2 * tap + 256],
                            start=(tap == 0), stop=False)
                    nc.tensor.matmul(
                        P[:], wh_t[0:74, wv, 4, m * 128:(m + 1) * 128],
                        r8[0:74, :, :], start=False, stop=True)

                S = y_pool.tile([128, W], F32)
                if g < 2:
                    # tanh-j path: S = [sig f; sig i]; CJ[64:] = tanh j;
                    # TO = [tanh c | sig o] on rows 0:64
                    TO = ts_pool.tile([64, 2 * W], F32)
                    nc.scalar.activation(out=CJ[g][64:128, :],
                                         in_=P1[0:64, :], func=ACTF.Tanh)
                    nc.scalar.activation(out=TO[:, W:2 * W],
                                         in_=P1[64:128, :],
                                         func=ACTF.Sigmoid)
                    nc.scalar.activation(out=S[:], in_=P0[:],
                                         func=ACTF.Sigmoid)
                    MM = m_pool.tile([64, 2 * W], F32)
                    nc.gpsimd.tensor_mul(MM[:, W:2 * W], S[64:128, :],
                                         CJ[g][64:128, :])
                    nc.vector.tensor_mul(MM[:, 0:W], S[0:64, :],
                                         CJ[g][0:64, :])
                    nc.vector.tensor_add(CJ[g][0:64, :], MM[:, 0:W],
                                         MM[:, W:2 * W])
                    nc.scalar.activation(out=TO[:, 0:W], in_=CJ[g][0:64, :],
                                         func=ACTF.Tanh)
                    nc.vector.tensor_mul(hnext[0:64, :, 4:260],
                                         TO[:, 0:W], TO[:, W:2 * W])
                    nc.vector.tensor_copy(out=hnext[64:128, :, 3:259],
                                          in_=hnext[0:64, :, 4:260])
                    # channel mean via PE ones-matmul, emitted 2 steps later
                else:
                    # sigmoid(2j) path: S1 = sig([2j; o]) in one ACT op;
                    # tanh j = 2*sig(2j)-1 on DVE; tanh c written to rows
                    # 64:128 so the Pool h'-mul has same-base inputs.
                    S1 = y_pool.tile([128, W], F32, name="S1", tag="s1")
                    T2 = ts_pool.tile([128, W], F32, name="T2", tag="t2")
                    nc.scalar.activation(out=S1[:], in_=P1[:],
                                         func=ACTF.Sigmoid)
                    nc.vector.tensor_scalar(CJ[g][64:128, :], S1[0:64, :],
                                            2.0, 1.0, ALU.mult, ALU.subtract)
                    nc.scalar.activation(out=S[:], in_=P0[:],
                                         func=ACTF.Sigmoid)
                    MM = m_pool.tile([64, 2 * W], F32)
                    nc.gpsimd.tensor_mul(MM[:, W:2 * W], S[64:128, :],
                                         CJ[g][64:128, :])
                    nc.vector.tensor_mul(MM[:, 0:W], S[0:64, :],
                                         CJ[g][0:64, :])
                    nc.vector.tensor_add(CJ[g][0:64, :], MM[:, 0:W],
                                         MM[:, W:2 * W])
                    nc.scalar.activation(out=T2[64:128, :],
                                         in_=CJ[g][0:64, :], func=ACTF.Tanh)
                    nc.gpsimd.tensor_mul(hnext[0:64, :, 4:260],
                                         T2[64:128, :], S1[64:128, :])
                    nc.vector.tensor_copy(out=hnext[64:128, :, 3:259],
                                          in_=hnext[0:64, :, 4:260])
                    srow = srow_pool.tile([64, W], F32)
                    nc.gpsimd.partition_all_reduce(
                        srow[:], hnext[0:64, :, 4:260], channels=64,
                        reduce_op=bass_isa.ReduceOp.add)
                    srow_prev[g] = srow   # flushed at the next step's top

                # stage the next step's K=74 rhs right after h is final, so
                # the in-order SP/DMA queue never stalls on unmet deps
                if t + 1 < T:
                    r8n = r8_pool.tile([80, 2, 256], F32R, name="r8n",
                                       tag="r8")
                    nc.sync.dma_start(out=r8n[64:74, :, :],
                                      in_=zpa_d[t + 1, g])
                    nc.sync.dma_start(out=r8n[0:64, :, :],
                                      in_=hnext[0:64, :, 8:264])
                    r8nxt[g] = r8n
            r8cur = r8nxt

        pe_means(T - 2)
        pe_means(T - 1)
        for g in range(2, NG):
            nc.sync.dma_start(
                out=outs_sb[64 + T - 1:64 + T, (g % 2) * W:(g % 2) * W + W],
                in_=srow_prev[g][0:1, :])

        outs_tb = consts.tile([128, 1024], F32)
        nc.scalar.activation(out=outs_tb[:], in_=outs_sb[:], func=ACTF.Tanh,
                             scale=1.0 / 64.0)
        for gh in range(2):
            nc.sync.dma_start(out=out_d[:, gh],
                              in_=outs_tb[64 * gh:64 * gh + 64, :])

    nc.compile()
    return nc


def _get_program():
    if 'nc' not in _CACHE:
        _CACHE['nc'] = _build_program()
    return _CACHE['nc']


def kernel(z, h0, c0, Wx, Wh, b):
    z = np.asarray(z, np.float32)
    h0 = np.asarray(h0, np.float32)
    c0 = np.asarray(c0, np.float32)
    whall = _prep_weights(np.asarray(Wx, np.float32),
                          np.asarray(Wh, np.float32),
                          np.asarray(b, np.float32))
    in_maps = []
    for core in range(NCORES):
        m = _prep_core(z, h0, c0, core)
        m['whall'] = whall
        in_maps.append(m)
    nc = _get_program()
    res = run_bass_kernel_spmd(nc, in_maps, list(range(NCORES)))
    outs = []
    for core in range(NCORES):
        R = res.results[core]['out']        # (64, 2, 2, 2, 256) [t,gh,gl,bb,f]
        outs.append(R.transpose(1, 2, 3, 0, 4).reshape(BL, T * F))
    return np.concatenate(outs, axis=0)


# revision 9
# speedup vs baseline: 1.6796x; 1.0946x over previous
"""ConvLSTM (Conv1D-LSTM over frames, sequential in time) on 8 NeuronCores.

Data-parallel over batch (8 per core). Per core the LSTM state is kept
transposed as (C=64, batch*frame) in SBUF so the recurrent 1-D conv becomes
PSUM-accumulated float32r matmuls with no per-step transpose:

  - 4 independent recurrent chains per core (one 2-batch group each, 512 gate
    columns) keep the PE densely fed (HAM-warm) and hide the per-step
    dependency chain.
  - h is stored padded (264 cols/batch); rows 64:128 hold a shift-by-1 copy
    so two conv taps contract per K=128 matmul (4 paired-tap matmuls).
  - the 9th tap, the z-conv (Cin=1), the bias, and the forget bias fold into
    one K=74 matmul over [h-slice; z-sliding-window; ones].

Engine rebalance vs the first version (Pool was at 94%, ACT 91%):
  - the K=74 h-slice copy is one SBUF-to-SBUF DMA (was DVE+Pool copies);
    it reads the previous step's h so the DMA latency is fully hidden.
  - groups 2/3 fold tanh(j) into the [j;o] sigmoid (j weights pre-scaled by
    2; tanh(j)=2*sigmoid(2j)-1 via one DVE tensor_scalar), saving one ACT op
    per group-step, and run the h'-multiply on Pool instead of DVE.
  - groups 0/1 compute the channel mean as a ones-vector PE matmul into a
    PSUM bank (emitted one step late so it never blocks gate matmuls in the
    in-order PE queue); groups 2/3 keep gpsimd partition_all_reduce.
"""
import sys
from contextlib import ExitStack

import numpy as np

if '/opt/trn_rl_repo' not in sys.path:
    sys.path.insert(0, '/opt/trn_rl_repo')

import concourse.bacc as bacc
import concourse.tile as tile
from concourse import bass_isa, mybir
from concourse.bass_utils import run_bass_kernel_spmd

B, T, F, C = 64, 64, 256, 64
NCORES, BL = 8, 8            # batches per core
NG = 4                       # groups (chains) per core, 2 batches each
COLS = F + 8                 # padded frame axis (4 each side)
W = 2 * F                    # free width per group (2 batches x 256)
F32 = mybir.dt.float32
F32R = mybir.dt.float32r
ACTF = mybir.ActivationFunctionType
ALU = mybir.AluOpType

_CACHE = {}


def _prep_weights(Wx, Wh, b):
    # gate reorder: [f, i, j, o]  (reference order: i, j, f, o)
    perm = np.concatenate([np.arange(128, 192), np.arange(0, 64),
                           np.arange(64, 128), np.arange(192, 256)])
    Whp = Wh[:, :, perm].astype(np.float32)            # (9, 64, 256)
    Wxp = Wx[:, 0, perm].astype(np.float32)            # (9, 256)
    bp = b[perm].astype(np.float32).copy()
    bp[0:64] += 1.0                                    # forget-gate bias
    whall = np.zeros((2, 128, 5, 256), np.float32)
    for p in range(4):
        whall[0, 0:64, p] = Whp[2 * p]
        whall[0, 64:128, p] = Whp[2 * p + 1]
    whall[0, 0:64, 4] = Whp[8]
    whall[0, 64:73, 4] = Wxp
    whall[0, 73, 4] = bp
    # variant 1 (groups 2/3): j columns scaled by 2 for the sigmoid(2j) trick
    whall[1] = whall[0]
    whall[1, :, :, 128:192] *= 2.0
    return np.ascontiguousarray(whall)


def _prep_core(z, h0, c0, core):
    zc = z[BL * core:BL * core + BL, :, :, 0]          # (8, T, F)
    h0c = h0[BL * core:BL * core + BL]                 # (8, F, C)
    c0c = c0[BL * core:BL * core + BL]

    zp = np.zeros((BL, T, COLS), np.float32)
    zp[:, :, 4:260] = zc
    zpa = np.ones((T, NG, 10, 2, 256), np.float32)
    for g in range(NG):
        for bb in range(2):
            bidx = 2 * g + bb
            for k in range(9):
                zpa[:, g, k, bb, :] = zp[bidx, :, k:k + 256]

    h0T = np.ascontiguousarray(h0c.transpose(2, 0, 1)).astype(np.float32)
    hh0 = np.zeros((2, NG, 128, 2, COLS), np.float32)
    for g in range(NG):
        for bb in range(2):
            hh0[0, g, 0:64, bb, 4:260] = h0T[:, 2 * g + bb, :]
    hh0[0, :, 64:128, :, 0:COLS - 1] = hh0[0, :, 0:64, :, 1:COLS]

    c0a = np.zeros((NG, 64, 2, 256), np.float32)
    for g in range(NG):
        for bb in range(2):
            c0a[g, :, bb, :] = c0c[2 * g + bb].T

    return {
        'zpa': np.ascontiguousarray(zpa.reshape(T, NG, 10, W)),
        'hh0': np.ascontiguousarray(hh0.reshape(2, NG, 128, 2 * COLS)),
        'c0a': np.ascontiguousarray(c0a.reshape(NG, 64, W)),
    }


def _build_program():
    nc = bacc.Bacc("TRN2", target_bir_lowering=False, debug=False,
                   enable_asserts=True, num_devices=NCORES)
    zpa_d = nc.dram_tensor("zpa", (T, NG, 10, W), F32R, kind="ExternalInput")
    hh0_d = nc.dram_tensor("hh0", (2, NG, 128, 2 * COLS), F32R,
                           kind="ExternalInput")
    c0a_d = nc.dram_tensor("c0a", (NG, 64, W), F32, kind="ExternalInput")
    wh_d = nc.dram_tensor("whall", (2, 128, 5, 256), F32R,
                          kind="ExternalInput")
    out_d = nc.dram_tensor("out", (64, 2, 2, 2, 256), F32, kind="ExternalOutput")

    with tile.TileContext(nc) as tc, ExitStack() as ctx:
        consts = ctx.enter_context(tc.tile_pool(name="consts", bufs=1))
        state = ctx.enter_context(tc.tile_pool(name="state", bufs=1))
        y_pool = ctx.enter_context(tc.tile_pool(name="ypool", bufs=4))
        ts_pool = ctx.enter_context(tc.tile_pool(name="tspool", bufs=4))
        m_pool = ctx.enter_context(tc.tile_pool(name="mpool", bufs=4))
        r8_pool = ctx.enter_context(tc.tile_pool(name="r8pool", bufs=12))
        srow_pool = ctx.enter_context(tc.tile_pool(name="srowpool", bufs=8))
        pg_pool = ctx.enter_context(tc.tile_pool(name="pgpool", bufs=7,
                                                 space="PSUM"))
        mb_pool = ctx.enter_context(tc.tile_pool(name="mbpool", bufs=1,
                                                 space="PSUM"))
        outs_pool = ctx.enter_context(tc.tile_pool(name="outs", bufs=1))

        wh_t = consts.tile([128, 2, 5, 256], F32R)
        nc.sync.dma_start(out=wh_t[:], in_=wh_d[:])
        ones64 = consts.tile([64, 1], F32R)
        nc.vector.memset(ones64[:], 1.0)

        hh = [[state.tile([128, 2, COLS], F32R, name=f"hh{par}{g}",
                          tag=f"hh{par}{g}")
               for g in range(NG)] for par in range(2)]
        # CJ[g]: rows 0:64 = c state (persistent), rows 64:128 = tanh(j)
        CJ = [state.tile([128, W], F32, name=f"CJ{g}", tag=f"CJ{g}")
              for g in range(NG)]
        for par in range(2):
            for g in range(NG):
                nc.sync.dma_start(out=hh[par][g][:], in_=hh0_d[par, g])
        for g in range(NG):
            nc.sync.dma_start(out=CJ[g][0:64, :], in_=c0a_d[g])

        outs_sb = outs_pool.tile([128, 1024], F32)
        MB = mb_pool.tile([128, 512], F32)   # PE-mean bank (groups 0/1)

        def pe_means(t):
            # channel means for groups 0/1 of step t, read from the parity
            # buffer h(t) lives in. Emitted two steps later so the matmuls
            # never stall the in-order PE queue behind fresh h updates.
            for g in range(2):
                nc.tensor.matmul(MB[32 * g:32 * g + 1, :], ones64[:],
                                 hh[(t + 1) % 2][g][0:64, :, 4:260],
                                 start=True, stop=True,
                                 tile_position=(0, 32 * g))
            sr2 = srow_pool.tile([2, W], F32, name="sr2", tag="sr2")
            nc.vector.tensor_copy(out=sr2[:], in_=MB[0:64:32, :])
            nc.sync.dma_start(out=outs_sb[t:t + 1, 0:1024], in_=sr2[:])

        # K=74 rhs tiles for step 0: z windows + h0-slice (dep: hh0 load)
        r8cur = []
        for g in range(NG):
            r8 = r8_pool.tile([80, 2, 256], F32R, name="r8p", tag="r8")
            nc.sync.dma_start(out=r8[64:74, :, :], in_=zpa_d[0, g])
            nc.sync.dma_start(out=r8[0:64, :, :], in_=hh[0][g][0:64, :, 8:264])
            r8cur.append(r8)
        srow_prev = [None] * NG

        for t in range(T):
            par, npar = t % 2, (t + 1) % 2
            if t >= 2:
                pe_means(t - 2)
            for g in range(2, NG):
                if srow_prev[g] is not None:
                    nc.sync.dma_start(
                        out=outs_sb[64 + t - 1:64 + t,
                                    (g % 2) * W:(g % 2) * W + W],
                        in_=srow_prev[g][0:1, :])
            r8nxt = [None] * NG
            for g in range(NG):
                wv = 0 if g < 2 else 1
                hcur, hnext = hh[par][g], hh[npar][g]
                r8 = r8cur[g]

                # P1 ([j; o]) first: its consumers get a head start.
                P1 = pg_pool.tile([128, W], F32, name="P1", tag="pg")
                P0 = pg_pool.tile([128, W], F32, name="P0", tag="pg")
                for m, P in ((1, P1), (0, P0)):
                    for tap in range(4):
                        nc.tensor.matmul(
                            P[:], wh_t[:, wv, tap, m * 128:(m + 1) * 128],
                            hcur[:, :, 2 * tap:2 * tap + 256],
                            start=(tap == 0), stop=False)
                    nc.tensor.matmul(
                        P[:], wh_t[0:74, wv, 4, m * 128:(m + 1) * 128],
                        r8[0:74, :, :], start=False, stop=True)

                S = y_pool.tile([128, W], F32)
                if g < 2:
                    # tanh-j path: S = [sig f; sig i]; CJ[64:] = tanh j;
                    # TO = [tanh c | sig o] on rows 0:64
                    TO = ts_pool.tile([64, 2 * W], F32)
                    nc.scalar.activation(out=CJ[g][64:128, :],
                                         in_=P1[0:64, :], func=ACTF.Tanh)
                    nc.scalar.activation(out=TO[:, W:2 * W],
                                         in_=P1[64:128, :],
                                         func=ACTF.Sigmoid)
                    nc.scalar.activation(out=S[:], in_=P0[:],
                                         func=ACTF.Sigmoid)
                    MM = m_pool.tile([64, 2 * W], F32)
                    nc.gpsimd.tensor_mul(MM[:, W:2 * W], S[64:128, :],
                                         CJ[g][64:128, :])
                    nc.vector.tensor_mul(MM[:, 0:W], S[0:64, :],
                                         CJ[g][0:64, :])
                    nc.vector.tensor_add(CJ[g][0:64, :], MM[:, 0:W],
                                         MM[:, W:2 * W])
                    nc.scalar.activation(out=TO[:, 0:W], in_=CJ[g][0:64, :],
                                         func=ACTF.Tanh)
                    nc.vector.tensor_mul(hnext[0:64, :, 4:260],
                                         TO[:, 0:W], TO[:, W:2 * W])
                    nc.vector.tensor_copy(out=hnext[64:128, :, 3:259],
                                          in_=hnext[0:64, :, 4:260])
                    # channel mean via PE ones-matmul, emitted 2 steps later
                else:
                    # sigmoid(2j) path: S1 = sig([2j; o]) in one ACT op;
                    # tanh j = 2*sig(2j)-1 on DVE; tanh c written to rows
                    # 64:128 so the Pool h'-mul has same-base inputs.
                    S1 = y_pool.tile([128, W], F32, name="S1", tag="s1")
                    T2 = ts_pool.tile([128, W], F32, name="T2", tag="t2")
                    nc.scalar.activation(out=S1[:], in_=P1[:],
                                         func=ACTF.Sigmoid)
                    nc.gpsimd.tensor_scalar(CJ[g][64:128, :], S1[0:64, :],
                                            2.0, 1.0, ALU.mult, ALU.subtract)
                    nc.scalar.activation(out=S[:], in_=P0[:],
                                         func=ACTF.Sigmoid)
                    MM = m_pool.tile([64, 2 * W], F32)
                    nc.vector.tensor_mul(MM[:, W:2 * W], S[64:128, :],
                                         CJ[g][64:128, :])
                    nc.gpsimd.tensor_mul(MM[:, 0:W], S[0:64, :],
                                         CJ[g][0:64, :])
                    nc.vector.tensor_add(CJ[g][0:64, :], MM[:, 0:W],
                                         MM[:, W:2 * W])
                    nc.scalar.activation(out=T2[64:128, :],
                                         in_=CJ[g][0:64, :], func=ACTF.Tanh)
                    nc.vector.tensor_mul(hnext[0:64, :, 4:260],
                                         T2[64:128, :], S1[64:128, :])
                    nc.vector.tensor_copy(out=hnext[64:128, :, 3:259],
                                          in_=hnext[0:64, :, 4:260])
                    srow = srow_pool.tile([64, W], F32)
                    nc.gpsimd.partition_all_reduce(
                        srow[:], hnext[0:64, :, 4:260], channels=64,
                        reduce_op=bass_isa.ReduceOp.add)
                    srow_prev[g] = srow   # flushed at the next step's top

                # stage the next step's K=74 rhs right after h is final, so
                # the in-order SP/DMA queue never stalls on unmet deps
                if t + 1 < T:
                    r8n = r8_pool.tile([80, 2, 256], F32R, name="r8n",
                                       tag="r8")
                    nc.sync.dma_start(out=r8n[64:74, :, :],
                                      in_=zpa_d[t + 1, g])
                    nc.sync.dma_start(out=r8n[0:64, :, :],
                                      in_=hnext[0:64, :, 8:264])
                    r8nxt[g] = r8n
            r8cur = r8nxt

        pe_means(T - 2)
        pe_means(T - 1)
        for g in range(2, NG):
            nc.sync.dma_start(
                out=outs_sb[64 + T - 1:64 + T, (g % 2) * W:(g % 2) * W + W],
                in_=srow_prev[g][0:1, :])

        outs_tb = consts.tile([128, 1024], F32)
        nc.scalar.activation(out=outs_tb[:], in_=outs_sb[:], func=ACTF.Tanh,
                             scale=1.0 / 64.0)
        for gh in range(2):
            nc.sync.dma_start(out=out_d[:, gh],
                              in_=outs_tb[64 * gh:64 * gh + 64, :])

    nc.compile()
    return nc


def _get_program():
    if 'nc' not in _CACHE:
        _CACHE['nc'] = _build_program()
    return _CACHE['nc']


def kernel(z, h0, c0, Wx, Wh, b):
    z = np.asarray(z, np.float32)
    h0 = np.asarray(h0, np.float32)
    c0 = np.asarray(c0, np.float32)
    whall = _prep_weights(np.asarray(Wx, np.float32),
                          np.asarray(Wh, np.float32),
                          np.asarray(b, np.float32))
    in_maps = []
    for core in range(NCORES):
        m = _prep_core(z, h0, c0, core)
        m['whall'] = whall
        in_maps.append(m)
    nc = _get_program()
    res = run_bass_kernel_spmd(nc, in_maps, list(range(NCORES)))
    outs = []
    for core in range(NCORES):
        R = res.results[core]['out']        # (64, 2, 2, 2, 256) [t,gh,gl,bb,f]
        outs.append(R.transpose(1, 2, 3, 0, 4).reshape(BL, T * F))
    return np.concatenate(outs, axis=0)
